# revision 11
# baseline (speedup 1.0000x reference)
"""Trainium2 Bass kernel for nn_AttentionEnhancedBiLSTM (8 NeuronCores, SPMD).

Math (from the reference), with the attention weights folded on the host:
    x  = inputs[:, -1, :]                               # [B=1024, E=1024]
    scores = x (Wq^T Wk / 32) x^T + w[None, :]          # Ms = Wq^T Wk / 32
    a  = softmax(scores)
    af = a x (Wo Wv)^T = (a @ x) @ N^T                  # N = Wo Wv
    h/c = lstm_cell((af + x + r) W_ih^T + b)            # only live gates kept
The backward direction's feature flip x[:, ::-1] is folded into the host
weights (Ms[::-1, ::-1], etc.), so both directions read the same x / x^T.
Attention biases reduce to the per-column score bias w = x Wk^T bq / 32 and
a constant row r = Wo bv + bo added to the residual (host-folded into x).

Sharding: batch-sharded 8 ways (128 rows/core), fully collective-free:
re-associating a @ (x N^T) as (a @ x) @ N^T lets every core work only on its
own 128 score rows while contracting over the full batch with the replicated
x it already holds for the scores matmul. All matmul operands are fp16
(full PE rate, 8x the mantissa of bf16), folded weights are half the bytes
of the originals; per-core HBM traffic is ~18 MiB vs ~60 MiB naive.

Schedule: all weights prefetch up-front on the two HW DGE queues in
consumption order (SBUF holds them all), and the two directions are emitted
interleaved stage-by-stage so each direction's matmuls fill the other's
PSUM-copy / transpose / softmax latency on the PE. exp() skips the max
shift (scores are O(1) by construction; f32 exp cannot overflow).
"""

import numpy as np

import concourse.bass as bass
import concourse.mybir as mybir
import concourse.tile as tile
from concourse import bacc
from concourse.bass_utils import run_bass_kernel_spmd
from concourse.masks import make_identity

N_CORES = 8
B, T, E, H = 1024, 128, 1024, 512
BS = B // N_CORES          # 128 batch rows per core
NE = E // 128              # 8 e-chunks
F32 = mybir.dt.float32
F16 = mybir.dt.float16
F16NP = np.float16


class _Dir:
    def __init__(self, d, ext, compute_h):
        self.d = d
        self.ext = ext
        self.compute_h = compute_h
        self.G = 3 * H if compute_h else 2 * H


def _emit(tc, nc, sb, ps, ident, ones, xo, xTf, xn, dirs, out_sb, out_ext,
          with_attn_bias):

    def w_load(w_ext, Gout, name, dma_eng, n_dmas):
        """Prefetch [E, Gout] weights as n_dmas chunk-major tiles."""
        tiles = []
        rows = E // n_dmas
        per = rows // 128                      # 128-row chunks per tile
        for piece in range(n_dmas):
            wt = sb.tile([128, per * Gout], F16, name=f"w_{name}_{piece}",
                         tag=f"w_{name}")
            dma_eng.dma_start(
                wt[:],
                w_ext[piece * rows:(piece + 1) * rows, :]
                .rearrange("(n p) m -> p n m", p=128))
            tiles.append(wt)

        def chunk(ec, n):                      # [128, 512] rhs slice
            t = tiles[ec // per]
            c = ec % per
            return t[:, c * Gout + n * 512: c * Gout + (n + 1) * 512]
        return chunk

    def mm_acc(lhsT_chunk, w_chunk, Gout, name, last_stop=True):
        acc = ps.tile([128, Gout], F32, name=f"ps_{name}", tag="mm")
        for ec in range(NE):
            for n in range(Gout // 512):
                nc.tensor.matmul(
                    acc[:, n * 512:(n + 1) * 512],
                    lhsT_chunk(ec),
                    w_chunk(ec, n),
                    start=(ec == 0),
                    stop=(ec == NE - 1 and last_stop),
                )
        return acc

    def add_bias_rows(acc, b_sb, Gout):
        """acc[128, Gout] += ones^T @ b (rank-1 broadcast of a bias row)."""
        for n in range(Gout // 512):
            nc.tensor.matmul(
                acc[:, n * 512:(n + 1) * 512],
                ones[0:1, :],
                b_sb[0:1, n * 512:(n + 1) * 512],
                start=False, stop=True,
            )

    def pe_transpose(src_sb, dst_name, dst_tag="act2"):
        """[128, 1024] natural fp16 -> [128, (ec, b)] transposed chunks."""
        out = sb.tile([128, E], F16, name=dst_name, tag=dst_tag)
        for half in range(2):
            tp = ps.tile([128, 512], F16, name=f"tp_{dst_name}_{half}",
                         tag="tp")
            for i in range(4):
                j = half * 4 + i
                nc.tensor.transpose(
                    tp[:, i * 128:(i + 1) * 128],
                    src_sb[:, j * 128:(j + 1) * 128],
                    ident[:],
                )
            nc.vector.tensor_copy(out[:, half * 512:(half + 1) * 512], tp[:])
        return out

    def psum_to_sb(acc, name, tag="act", dt=F16):
        out = sb.tile([128, E], dt, name=name, tag=tag)
        for n in range(2):
            nc.vector.tensor_copy(out[:, n * 512:(n + 1) * 512],
                                  acc[:, n * 512:(n + 1) * 512])
        return out

    xo_chunk = lambda ec: xo[:, ec * BS:(ec + 1) * BS]

    # ---- weight prefetch, in consumption order ---------------------------
    for st in dirs:
        st.ms = w_load(st.ext["ms"], E, f"ms{st.d}", nc.scalar,
                       4 if st.d == "f" else 2)
    for st in dirs:
        st.nv = w_load(st.ext["nv"], E, f"nv{st.d}", nc.sync, 2)
    for st in dirs:
        st.wih = w_load(st.ext["wih"], st.G, f"wih{st.d}", nc.scalar, 2)
    for st in dirs:
        st.xe_sb = sb.tile([128, E], F16, name=f"xe_{st.d}", tag="xe")
        nc.sync.dma_start(st.xe_sb[:], st.ext["xe"][:])
        st.bih = sb.tile([1, st.G], F16, name=f"bih_{st.d}", tag="bias")
        nc.sync.dma_start(st.bih[:], st.ext["bih"][:])
        if with_attn_bias:
            st.wrow = sb.tile([1, B], F16, name=f"wrow_{st.d}", tag="bias")
            nc.sync.dma_start(st.wrow[:], st.ext["w"][:])

    # ---- phase B (interleaved dirs): scores + softmax --------------------
    for st in dirs:
        st.s_ps = mm_acc(xo_chunk, st.ms, E, f"s{st.d}")
    for st in dirs:
        st.s_sb = psum_to_sb(st.s_ps, f"s_{st.d}")
    for st in dirs:
        st.sT = pe_transpose(st.s_sb, f"sT_{st.d}")
    for st in dirs:
        scores = ps.tile([128, B], F32, name=f"scores_{st.d}", tag="mm")
        for ec in range(NE):
            for n in range(B // 512):
                nc.tensor.matmul(
                    scores[:, n * 512:(n + 1) * 512],
                    st.sT[:, ec * 128:(ec + 1) * 128],
                    xTf[:, ec * B + n * 512: ec * B + (n + 1) * 512],
                    start=(ec == 0), stop=(ec == NE - 1 and not with_attn_bias),
                )
        if with_attn_bias:
            add_bias_rows(scores, st.wrow, B)
        # scores are O(1) by construction -> exp() directly, no max shift
        st.p_sb = sb.tile([128, B], F16, name=f"p_{st.d}", tag="act")
        rowsum = sb.tile([128, 1], F32, name=f"rowsum_{st.d}", tag="stat")
        nc.scalar.activation(st.p_sb[:], scores[:],
                             mybir.ActivationFunctionType.Exp,
                             scale=1.0, accum_out=rowsum[:])
        st.rinv = sb.tile([128, 1], F32, name=f"rinv_{st.d}", tag="stat")
        nc.vector.reciprocal(st.rinv[:], rowsum[:])
    for st in dirs:
        st.pT = pe_transpose(st.p_sb, f"pT_{st.d}")

    # ---- phase C (interleaved dirs): af = (p @ x) @ N^T + LSTM cell ------
    for st in dirs:
        st.px_ps = ps.tile([128, E], F32, name=f"px_{st.d}", tag="mm")
        for bc in range(NE):
            for n in range(E // 512):
                nc.tensor.matmul(
                    st.px_ps[:, n * 512:(n + 1) * 512],
                    st.pT[:, bc * 128:(bc + 1) * 128],
                    xn[:, bc * E + n * 512: bc * E + (n + 1) * 512],
                    start=(bc == 0), stop=(bc == NE - 1),
                )
    for st in dirs:
        st.px_sb = psum_to_sb(st.px_ps, f"px_{st.d}")
    for st in dirs:
        st.pxT = pe_transpose(st.px_sb, f"pxT_{st.d}")
    for st in dirs:
        st.av_ps = mm_acc(
            lambda ec: st.pxT[:, ec * 128:(ec + 1) * 128],
            st.nv, E, f"av{st.d}")
    for st in dirs:
        # lstm_in = av * rinv + x_eff, in 512-halves to overlap downstream
        st.lstm_sb = sb.tile([128, E], F16, name=f"lstm_{st.d}", tag="act")
        for n in range(2):
            hv = slice(n * 512, (n + 1) * 512)
            av_n = sb.tile([128, 512], F32, name=f"avn_{st.d}_{n}", tag="avn")
            nc.vector.tensor_scalar_mul(av_n[:], st.av_ps[:, hv], st.rinv[:])
            nc.vector.tensor_add(st.lstm_sb[:, hv], av_n[:], st.xe_sb[:, hv])
    for st in dirs:
        st.lstmT = pe_transpose(st.lstm_sb, f"lstmT_{st.d}")
    for st in dirs:
        st.gates = mm_acc(
            lambda ec: st.lstmT[:, ec * 128:(ec + 1) * 128],
            st.wih, st.G, f"g{st.d}", last_stop=False)
        add_bias_rows(st.gates, st.bih, st.G)

    Sig = mybir.ActivationFunctionType.Sigmoid
    Tanh = mybir.ActivationFunctionType.Tanh
    for st in dirs:
        d, G, gates = st.d, st.G, st.gates
        si = sb.tile([128, H], F32, name=f"si_{d}", tag="gate")
        nc.scalar.activation(si[:], gates[:, 0:H], Sig)
        tg = sb.tile([128, H], F32, name=f"tg_{d}", tag="gate")
        nc.scalar.activation(tg[:], gates[:, H:2 * H], Tanh)
        if st.compute_h:
            cst = sb.tile([128, H], F32, name=f"c_{d}", tag="gate")
            nc.vector.tensor_mul(cst[:], si[:], tg[:])
            tc_ = sb.tile([128, H], F32, name=f"tc_{d}", tag="gate")
            nc.scalar.activation(tc_[:], cst[:], Tanh)
            so = sb.tile([128, H], F32, name=f"so_{d}", tag="gate")
            nc.scalar.activation(so[:], gates[:, 2 * H:3 * H], Sig)
            nc.vector.tensor_mul(out_sb[:, 0:H], so[:], tc_[:])
            nc.sync.dma_start(out_ext[:, 0:H], out_sb[:, 0:H])
        else:
            nc.vector.tensor_mul(out_sb[:, H:2 * H], si[:], tg[:])
            nc.sync.dma_start(out_ext[:, H:2 * H], out_sb[:, H:2 * H])


def build_nc(with_attn_bias=False):
    nc = bacc.Bacc("TRN2", target_bir_lowering=False, debug=False,
                   num_devices=N_CORES)

    def din(name, shape, dt=F16):
        return nc.dram_tensor(name, shape, dt, kind="ExternalInput").ap()

    ext = {}
    for d in ("f", "b"):
        G = 3 * H if d == "f" else 2 * H
        ext[d] = {
            "ms": din(f"ms_{d}", [E, E]),
            "nv": din(f"nv_{d}", [E, E]),
            "wih": din(f"wih_{d}", [E, G]),
            "bih": din(f"bih_{d}", [1, G]),
            "w": din(f"w_{d}", [1, B]),
            "xe": din(f"xe_{d}", [BS, E]),
        }
    xTo_ext = din("xTo", [E, BS])
    xTf_ext = din("xTf", [E, B])
    xn_ext = din("xn", [B, E])
    out_ext = nc.dram_tensor("out", [BS, 2 * H], F32, kind="ExternalOutput").ap()

    with tile.TileContext(nc) as tc:
        with (
            tc.tile_pool(name="sb", bufs=1) as sb_pool,
            tc.tile_pool(name="ps", bufs=1, space="PSUM") as ps_pool,
        ):
            class P:
                def __init__(self, pool, defaults):
                    self.pool, self.defaults = pool, defaults

                def tile(self, shape, dtype, name=None, tag=""):
                    bufs = self.defaults.get(tag, 1)
                    return self.pool.tile(shape, dtype, name=name, tag=tag,
                                          bufs=bufs)

            sb = P(sb_pool, {"act": 4, "act2": 4, "bias": 4,
                             "gate": 6, "stat": 4, "avn": 4, "xe": 2})
            ps = P(ps_pool, {"mm": 2, "tp": 2})

            ident_f = sb_pool.tile([128, 128], F32, name="ident_f",
                                   tag="ident_f")
            make_identity(nc, ident_f)
            ident = sb_pool.tile([128, 128], F16, name="ident", tag="ident")
            nc.vector.tensor_copy(ident[:], ident_f[:])
            ones_f = sb_pool.tile([1, 128], F32, name="ones_f", tag="ones_f")
            nc.gpsimd.memset(ones_f[:], 1.0)
            ones = sb_pool.tile([1, 128], F16, name="ones", tag="ones")
            nc.vector.tensor_copy(ones[:], ones_f[:])

            xo = sb_pool.tile([128, E], F16, name="xo", tag="xo")
            nc.sync.dma_start(xo[:],
                              xTo_ext.rearrange("(n p) m -> p n m", p=128))
            xTf = sb_pool.tile([128, NE * B], F16, name="xTf", tag="xTf")
            for q in range(4):
                nc.sync.dma_start(
                    xTf[:, q * 2 * B:(q + 1) * 2 * B],
                    xTf_ext[q * 256:(q + 1) * 256, :]
                    .rearrange("(n p) m -> p n m", p=128))
            xn = sb_pool.tile([128, NE * E], F16, name="xn", tag="xn")
            for q in range(2):
                nc.sync.dma_start(
                    xn[:, q * 4 * E:(q + 1) * 4 * E],
                    xn_ext[q * 512:(q + 1) * 512, :]
                    .rearrange("(g p) m -> p g m", p=128))

            out_sb = sb_pool.tile([BS, 2 * H], F32, name="out_sb", tag="out")

            dirs = [_Dir("f", ext["f"], True), _Dir("b", ext["b"], False)]
            _emit(tc, nc, sb, ps, ident, ones, xo, xTf, xn, dirs, out_sb,
                  out_ext, with_attn_bias)

    nc.compile()
    return nc


_NC_CACHE = {}


def _get_nc(with_attn_bias=False):
    if with_attn_bias not in _NC_CACHE:
        _NC_CACHE[with_attn_bias] = build_nc(with_attn_bias)
    return _NC_CACHE[with_attn_bias]


def _fold_dir(x, Wqkv, bqkv, Wo, bo, W_ih, b_ih, b_hh, flip):
    """Host-side weight folding for one direction. Returns f32 arrays."""
    c = np.ascontiguousarray
    Wq, Wk, Wv = Wqkv[0:E], Wqkv[E:2 * E], Wqkv[2 * E:3 * E]
    bq, bv = bqkv[0:E], bqkv[2 * E:3 * E]
    Ms = (Wq.T @ Wk) / 32.0                      # scores = x Ms x^T + w
    N = (Wo @ Wv).T                              # v' = x N  (rhs layout)
    r = Wo @ bv + bo                             # row bias folded into x
    gsel = (0, 2, 3) if not flip else (0, 2)     # live gates (i, g[, o])
    wih = np.concatenate([W_ih[g * H:(g + 1) * H] for g in gsel], 0).T
    blstm = b_ih + b_hh
    bih = np.concatenate([blstm[g * H:(g + 1) * H] for g in gsel])
    if flip:
        ms = Ms[::-1, ::-1]
        nv = N[::-1, ::-1]
        wih = wih[::-1, :]
        w = (x[:, ::-1] @ (Wk.T @ bq)) / 32.0
        xe = x + r[::-1][None, :]
    else:
        ms, nv = Ms, N
        w = x @ (Wk.T @ bq) / 32.0
        xe = x + r[None, :]
    return dict(ms=c(ms), nv=c(nv), wih=c(wih),
                bih=c(bih.reshape(1, -1)), w=c(w.reshape(1, B)), xe=xe)


def _prepare(inputs, Wqkv_f, bqkv_f, Wo_f, bo_f, W_ih_f, b_ih_f, b_hh_f,
             Wqkv_b, bqkv_b, Wo_b, bo_b, W_ih_b, b_ih_b, b_hh_b):
    f32 = lambda a: np.asarray(a, dtype=np.float32)
    x = np.ascontiguousarray(f32(inputs)[:, -1, :])          # [B, E]

    with_attn_bias = bool(
        np.any(f32(bqkv_f)) or np.any(f32(bo_f))
        or np.any(f32(bqkv_b)) or np.any(f32(bo_b)))

    folds = {
        "f": _fold_dir(x, f32(Wqkv_f), f32(bqkv_f), f32(Wo_f), f32(bo_f),
                       f32(W_ih_f), f32(b_ih_f), f32(b_hh_f), flip=False),
        "b": _fold_dir(x, f32(Wqkv_b), f32(bqkv_b), f32(Wo_b), f32(bo_b),
                       f32(W_ih_b), f32(b_ih_b), f32(b_hh_b), flip=True),
    }
    f16 = lambda a: np.ascontiguousarray(a.astype(F16NP))
    shared = {}
    for d, fo in folds.items():
        for k in ("ms", "nv", "wih", "bih", "w"):
            shared[f"{k}_{d}"] = f16(fo[k])
    xn16 = f16(x)
    xT16 = f16(x.T)

    in_maps = []
    for ci in range(N_CORES):
        rows = slice(ci * BS, (ci + 1) * BS)
        m = dict(shared)
        m["xTo"] = f16(np.ascontiguousarray(x[rows].T))
        m["xn"] = xn16
        m["xTf"] = xT16
        m["xe_f"] = f16(folds["f"]["xe"][rows])
        m["xe_b"] = f16(folds["b"]["xe"][rows])
        in_maps.append(m)
    return in_maps, with_attn_bias


def build_in_maps(inputs_dict):
    """Per-core input maps from the full input dict (for test harness reuse)."""
    return _prepare(**inputs_dict)[0]


def kernel(**inputs):
    in_maps, with_attn_bias = _prepare(**inputs)
    nc = _get_nc(with_attn_bias)
    res = run_bass_kernel_spmd(nc, in_maps, core_ids=list(range(N_CORES)))
    out = np.concatenate([res.results[ci]["out"] for ci in range(N_CORES)],
                         axis=0)
    return out.astype(np.float32)


# revision 12
# speedup vs baseline: 1.5924x; 1.5924x over previous
"""Trainium2 Bass kernel for nn_AttentionEnhancedBiLSTM (8 NeuronCores, SPMD).

Math (from the reference), with the attention weights folded on the host:
    x  = inputs[:, -1, :]                               # [B=1024, E=1024]
    scores = x (Wq^T Wk / 32) x^T + w[None, :]          # Ms = Wq^T Wk / 32
    a  = softmax(scores)
    af = a x (Wo Wv)^T = (a @ x) @ N^T                  # N = Wo Wv
    h/c = lstm_cell((af + x + r) W_ih^T + b)            # only live gates kept
The backward direction's feature flip x[:, ::-1] is folded into the host
weights (Ms[::-1, ::-1], etc.), so both directions read the same x / x^T.
Attention biases reduce to the per-column score bias w = x Wk^T bq / 32 and
a constant row r = Wo bv + bo added to the residual (host-folded into x).

Sharding: batch-sharded 8 ways (128 rows/core), fully collective-free:
re-associating a @ (x N^T) as (a @ x) @ N^T lets every core work only on its
own 128 score rows while contracting over the full batch with the replicated
x it already holds for the scores matmul. All matmul operands are fp16
(full PE rate, 8x the mantissa of bf16), folded weights are half the bytes
of the originals; per-core HBM traffic is ~18 MiB vs ~60 MiB naive.

Schedule: all weights prefetch up-front on the two HW DGE queues in
consumption order (SBUF holds them all), and the two directions are emitted
interleaved stage-by-stage so each direction's matmuls fill the other's
PSUM-copy / transpose / softmax latency on the PE. exp() skips the max
shift (scores are O(1) by construction; f32 exp cannot overflow).
"""

import numpy as np

import concourse.bass as bass
import concourse.mybir as mybir
import concourse.tile as tile
from concourse import bacc
from concourse.bass_utils import run_bass_kernel_spmd
from concourse.masks import make_identity

N_CORES = 8
B, T, E, H = 1024, 128, 1024, 512
BS = B // N_CORES          # 128 batch rows per core
NE = E // 128              # 8 e-chunks
F32 = mybir.dt.float32
F16 = mybir.dt.float16
F16NP = np.float16


class _Dir:
    def __init__(self, d, ext, compute_h):
        self.d = d
        self.ext = ext
        self.compute_h = compute_h
        self.G = 3 * H if compute_h else 2 * H


def _emit(tc, nc, sb, ps, ident, ones, xo, xTf, xn, dirs, out_sb, out_ext,
          with_attn_bias):

    def w_load(w_ext, Gout, name, dma_eng, n_dmas):
        """Prefetch [E, Gout] weights as n_dmas chunk-major tiles."""
        tiles = []
        rows = E // n_dmas
        per = rows // 128                      # 128-row chunks per tile
        for piece in range(n_dmas):
            wt = sb.tile([128, per * Gout], F16, name=f"w_{name}_{piece}",
                         tag=f"w_{name}_{piece}")
            dma_eng.dma_start(
                wt[:],
                w_ext[piece * rows:(piece + 1) * rows, :]
                .rearrange("(n p) m -> p n m", p=128))
            tiles.append(wt)

        def chunk(ec, n):                      # [128, 512] rhs slice
            t = tiles[ec // per]
            c = ec % per
            return t[:, c * Gout + n * 512: c * Gout + (n + 1) * 512]
        return chunk

    def mm_acc(lhsT_chunk, w_chunk, Gout, name, last_stop=True):
        acc = ps.tile([128, Gout], F32, name=f"ps_{name}", tag="mm")
        for ec in range(NE):
            for n in range(Gout // 512):
                nc.tensor.matmul(
                    acc[:, n * 512:(n + 1) * 512],
                    lhsT_chunk(ec),
                    w_chunk(ec, n),
                    start=(ec == 0),
                    stop=(ec == NE - 1 and last_stop),
                )
        return acc

    def add_bias_rows(acc, b_sb, Gout):
        """acc[128, Gout] += ones^T @ b (rank-1 broadcast of a bias row)."""
        for n in range(Gout // 512):
            nc.tensor.matmul(
                acc[:, n * 512:(n + 1) * 512],
                ones[0:1, :],
                b_sb[0:1, n * 512:(n + 1) * 512],
                start=False, stop=True,
            )

    def pe_transpose(src_sb, dst_name, dst_tag="act2"):
        """[128, 1024] natural fp16 -> [128, (ec, b)] transposed chunks."""
        out = sb.tile([128, E], F16, name=dst_name, tag=dst_tag)
        for half in range(2):
            tp = ps.tile([128, 512], F16, name=f"tp_{dst_name}_{half}",
                         tag="tp")
            for i in range(4):
                j = half * 4 + i
                nc.tensor.transpose(
                    tp[:, i * 128:(i + 1) * 128],
                    src_sb[:, j * 128:(j + 1) * 128],
                    ident[:],
                )
            nc.vector.tensor_copy(out[:, half * 512:(half + 1) * 512], tp[:])
        return out

    def psum_to_sb(acc, name, tag="act", dt=F16):
        out = sb.tile([128, E], dt, name=name, tag=tag)
        for n in range(2):
            nc.vector.tensor_copy(out[:, n * 512:(n + 1) * 512],
                                  acc[:, n * 512:(n + 1) * 512])
        return out

    xo_chunk = lambda ec: xo[:, ec * BS:(ec + 1) * BS]

    # ---- weight prefetch, in consumption order ---------------------------
    for st in dirs:
        st.ms = w_load(st.ext["ms"], E, f"ms{st.d}", nc.scalar,
                       4 if st.d == "f" else 2)
    for st in dirs:
        st.nv = w_load(st.ext["nv"], E, f"nv{st.d}", nc.sync, 2)
    for st in dirs:
        st.wih = w_load(st.ext["wih"], st.G, f"wih{st.d}", nc.scalar, 2)
    for st in dirs:
        st.xe_sb = sb.tile([128, E], F16, name=f"xe_{st.d}", tag="xe")
        nc.sync.dma_start(st.xe_sb[:], st.ext["xe"][:])
        st.bih = sb.tile([1, st.G], F16, name=f"bih_{st.d}", tag="bias")
        nc.sync.dma_start(st.bih[:], st.ext["bih"][:])
        if with_attn_bias:
            st.wrow = sb.tile([1, B], F16, name=f"wrow_{st.d}", tag="bias")
            nc.sync.dma_start(st.wrow[:], st.ext["w"][:])

    # ---- phase B (interleaved dirs): scores + softmax --------------------
    for st in dirs:
        st.s_ps = mm_acc(xo_chunk, st.ms, E, f"s{st.d}")
    for st in dirs:
        st.s_sb = psum_to_sb(st.s_ps, f"s_{st.d}")
    for st in dirs:
        st.sT = pe_transpose(st.s_sb, f"sT_{st.d}")
    for st in dirs:
        scores = ps.tile([128, B], F32, name=f"scores_{st.d}", tag="mm")
        for ec in range(NE):
            for n in range(B // 512):
                nc.tensor.matmul(
                    scores[:, n * 512:(n + 1) * 512],
                    st.sT[:, ec * 128:(ec + 1) * 128],
                    xTf[:, ec * B + n * 512: ec * B + (n + 1) * 512],
                    start=(ec == 0), stop=(ec == NE - 1 and not with_attn_bias),
                )
        if with_attn_bias:
            add_bias_rows(scores, st.wrow, B)
        # scores are O(1) by construction -> exp() directly, no max shift
        st.p_sb = sb.tile([128, B], F16, name=f"p_{st.d}", tag="act")
        rowsum = sb.tile([128, 1], F32, name=f"rowsum_{st.d}", tag="stat")
        nc.scalar.activation(st.p_sb[:], scores[:],
                             mybir.ActivationFunctionType.Exp,
                             scale=1.0, accum_out=rowsum[:])
        st.rinv = sb.tile([128, 1], F32, name=f"rinv_{st.d}", tag="stat")
        nc.vector.reciprocal(st.rinv[:], rowsum[:])
    for st in dirs:
        st.pT = pe_transpose(st.p_sb, f"pT_{st.d}")

    # ---- phase C (interleaved dirs): af = (p @ x) @ N^T + LSTM cell ------
    for st in dirs:
        st.px_ps = ps.tile([128, E], F32, name=f"px_{st.d}", tag="mm")
        for bc in range(NE):
            for n in range(E // 512):
                nc.tensor.matmul(
                    st.px_ps[:, n * 512:(n + 1) * 512],
                    st.pT[:, bc * 128:(bc + 1) * 128],
                    xn[:, bc * E + n * 512: bc * E + (n + 1) * 512],
                    start=(bc == 0), stop=(bc == NE - 1),
                )
    for st in dirs:
        st.px_sb = psum_to_sb(st.px_ps, f"px_{st.d}")
    for st in dirs:
        st.pxT = pe_transpose(st.px_sb, f"pxT_{st.d}")
    for st in dirs:
        st.av_ps = mm_acc(
            lambda ec: st.pxT[:, ec * 128:(ec + 1) * 128],
            st.nv, E, f"av{st.d}")
    for st in dirs:
        # lstm_in = av * rinv + x_eff, in 512-halves to overlap downstream
        st.lstm_sb = sb.tile([128, E], F16, name=f"lstm_{st.d}", tag="act")
        for n in range(2):
            hv = slice(n * 512, (n + 1) * 512)
            av_n = sb.tile([128, 512], F32, name=f"avn_{st.d}_{n}", tag="avn")
            nc.vector.tensor_scalar_mul(av_n[:], st.av_ps[:, hv], st.rinv[:])
            nc.vector.tensor_add(st.lstm_sb[:, hv], av_n[:], st.xe_sb[:, hv])
    for st in dirs:
        st.lstmT = pe_transpose(st.lstm_sb, f"lstmT_{st.d}")
    for st in dirs:
        st.gates = mm_acc(
            lambda ec: st.lstmT[:, ec * 128:(ec + 1) * 128],
            st.wih, st.G, f"g{st.d}", last_stop=False)
        add_bias_rows(st.gates, st.bih, st.G)

    Sig = mybir.ActivationFunctionType.Sigmoid
    Tanh = mybir.ActivationFunctionType.Tanh
    for st in dirs:
        d, G, gates = st.d, st.G, st.gates
        si = sb.tile([128, H], F32, name=f"si_{d}", tag="gate")
        nc.scalar.activation(si[:], gates[:, 0:H], Sig)
        tg = sb.tile([128, H], F32, name=f"tg_{d}", tag="gate")
        nc.scalar.activation(tg[:], gates[:, H:2 * H], Tanh)
        if st.compute_h:
            cst = sb.tile([128, H], F32, name=f"c_{d}", tag="gate")
            nc.vector.tensor_mul(cst[:], si[:], tg[:])
            tc_ = sb.tile([128, H], F32, name=f"tc_{d}", tag="gate")
            nc.scalar.activation(tc_[:], cst[:], Tanh)
            so = sb.tile([128, H], F32, name=f"so_{d}", tag="gate")
            nc.scalar.activation(so[:], gates[:, 2 * H:3 * H], Sig)
            nc.vector.tensor_mul(out_sb[:, 0:H], so[:], tc_[:])
            nc.sync.dma_start(out_ext[:, 0:H], out_sb[:, 0:H])
        else:
            nc.vector.tensor_mul(out_sb[:, H:2 * H], si[:], tg[:])
            nc.sync.dma_start(out_ext[:, H:2 * H], out_sb[:, H:2 * H])


def build_nc(with_attn_bias=False):
    nc = bacc.Bacc("TRN2", target_bir_lowering=False, debug=False,
                   num_devices=N_CORES)

    def din(name, shape, dt=F16):
        return nc.dram_tensor(name, shape, dt, kind="ExternalInput").ap()

    ext = {}
    for d in ("f", "b"):
        G = 3 * H if d == "f" else 2 * H
        ext[d] = {
            "ms": din(f"ms_{d}", [E, E]),
            "nv": din(f"nv_{d}", [E, E]),
            "wih": din(f"wih_{d}", [E, G]),
            "bih": din(f"bih_{d}", [1, G]),
            "w": din(f"w_{d}", [1, B]),
            "xe": din(f"xe_{d}", [BS, E]),
        }
    xTo_ext = din("xTo", [E, BS])
    xTf_ext = din("xTf", [E, B])
    xn_ext = din("xn", [B, E])
    out_ext = nc.dram_tensor("out", [BS, 2 * H], F32, kind="ExternalOutput").ap()

    with tile.TileContext(nc) as tc:
        with (
            tc.tile_pool(name="sb", bufs=1) as sb_pool,
            tc.tile_pool(name="ps", bufs=1, space="PSUM") as ps_pool,
        ):
            class P:
                def __init__(self, pool, defaults):
                    self.pool, self.defaults = pool, defaults

                def tile(self, shape, dtype, name=None, tag=""):
                    bufs = self.defaults.get(tag, 1)
                    return self.pool.tile(shape, dtype, name=name, tag=tag,
                                          bufs=bufs)

            sb = P(sb_pool, {"act": 4, "act2": 4, "bias": 4,
                             "gate": 6, "stat": 4, "avn": 4, "xe": 2})
            ps = P(ps_pool, {"mm": 2, "tp": 2})

            ident_f = sb_pool.tile([128, 128], F32, name="ident_f",
                                   tag="ident_f")
            make_identity(nc, ident_f)
            ident = sb_pool.tile([128, 128], F16, name="ident", tag="ident")
            nc.vector.tensor_copy(ident[:], ident_f[:])
            ones_f = sb_pool.tile([1, 128], F32, name="ones_f", tag="ones_f")
            nc.gpsimd.memset(ones_f[:], 1.0)
            ones = sb_pool.tile([1, 128], F16, name="ones", tag="ones")
            nc.vector.tensor_copy(ones[:], ones_f[:])

            xo = sb_pool.tile([128, E], F16, name="xo", tag="xo")
            nc.sync.dma_start(xo[:],
                              xTo_ext.rearrange("(n p) m -> p n m", p=128))
            xTf = sb_pool.tile([128, NE * B], F16, name="xTf", tag="xTf")
            for q in range(4):
                nc.sync.dma_start(
                    xTf[:, q * 2 * B:(q + 1) * 2 * B],
                    xTf_ext[q * 256:(q + 1) * 256, :]
                    .rearrange("(n p) m -> p n m", p=128))
            xn = sb_pool.tile([128, NE * E], F16, name="xn", tag="xn")
            for q in range(2):
                nc.sync.dma_start(
                    xn[:, q * 4 * E:(q + 1) * 4 * E],
                    xn_ext[q * 512:(q + 1) * 512, :]
                    .rearrange("(g p) m -> p g m", p=128))

            out_sb = sb_pool.tile([BS, 2 * H], F32, name="out_sb", tag="out")

            dirs = [_Dir("f", ext["f"], True), _Dir("b", ext["b"], False)]
            _emit(tc, nc, sb, ps, ident, ones, xo, xTf, xn, dirs, out_sb,
                  out_ext, with_attn_bias)

    nc.compile()
    return nc


_NC_CACHE = {}


def _get_nc(with_attn_bias=False):
    if with_attn_bias not in _NC_CACHE:
        _NC_CACHE[with_attn_bias] = build_nc(with_attn_bias)
    return _NC_CACHE[with_attn_bias]


def _fold_dir(x, Wqkv, bqkv, Wo, bo, W_ih, b_ih, b_hh, flip):
    """Host-side weight folding for one direction. Returns f32 arrays."""
    c = np.ascontiguousarray
    Wq, Wk, Wv = Wqkv[0:E], Wqkv[E:2 * E], Wqkv[2 * E:3 * E]
    bq, bv = bqkv[0:E], bqkv[2 * E:3 * E]
    Ms = (Wq.T @ Wk) / 32.0                      # scores = x Ms x^T + w
    N = (Wo @ Wv).T                              # v' = x N  (rhs layout)
    r = Wo @ bv + bo                             # row bias folded into x
    gsel = (0, 2, 3) if not flip else (0, 2)     # live gates (i, g[, o])
    wih = np.concatenate([W_ih[g * H:(g + 1) * H] for g in gsel], 0).T
    blstm = b_ih + b_hh
    bih = np.concatenate([blstm[g * H:(g + 1) * H] for g in gsel])
    if flip:
        ms = Ms[::-1, ::-1]
        nv = N[::-1, ::-1]
        wih = wih[::-1, :]
        w = (x[:, ::-1] @ (Wk.T @ bq)) / 32.0
        xe = x + r[::-1][None, :]
    else:
        ms, nv = Ms, N
        w = x @ (Wk.T @ bq) / 32.0
        xe = x + r[None, :]
    return dict(ms=c(ms), nv=c(nv), wih=c(wih),
                bih=c(bih.reshape(1, -1)), w=c(w.reshape(1, B)), xe=xe)


def _prepare(inputs, Wqkv_f, bqkv_f, Wo_f, bo_f, W_ih_f, b_ih_f, b_hh_f,
             Wqkv_b, bqkv_b, Wo_b, bo_b, W_ih_b, b_ih_b, b_hh_b):
    f32 = lambda a: np.asarray(a, dtype=np.float32)
    x = np.ascontiguousarray(f32(inputs)[:, -1, :])          # [B, E]

    with_attn_bias = bool(
        np.any(f32(bqkv_f)) or np.any(f32(bo_f))
        or np.any(f32(bqkv_b)) or np.any(f32(bo_b)))

    folds = {
        "f": _fold_dir(x, f32(Wqkv_f), f32(bqkv_f), f32(Wo_f), f32(bo_f),
                       f32(W_ih_f), f32(b_ih_f), f32(b_hh_f), flip=False),
        "b": _fold_dir(x, f32(Wqkv_b), f32(bqkv_b), f32(Wo_b), f32(bo_b),
                       f32(W_ih_b), f32(b_ih_b), f32(b_hh_b), flip=True),
    }
    f16 = lambda a: np.ascontiguousarray(a.astype(F16NP))
    shared = {}
    for d, fo in folds.items():
        for k in ("ms", "nv", "wih", "bih", "w"):
            shared[f"{k}_{d}"] = f16(fo[k])
    xn16 = f16(x)
    xT16 = f16(x.T)

    in_maps = []
    for ci in range(N_CORES):
        rows = slice(ci * BS, (ci + 1) * BS)
        m = dict(shared)
        m["xTo"] = f16(np.ascontiguousarray(x[rows].T))
        m["xn"] = xn16
        m["xTf"] = xT16
        m["xe_f"] = f16(folds["f"]["xe"][rows])
        m["xe_b"] = f16(folds["b"]["xe"][rows])
        in_maps.append(m)
    return in_maps, with_attn_bias


def build_in_maps(inputs_dict):
    """Per-core input maps from the full input dict (for test harness reuse)."""
    return _prepare(**inputs_dict)[0]


def kernel(**inputs):
    in_maps, with_attn_bias = _prepare(**inputs)
    nc = _get_nc(with_attn_bias)
    res = run_bass_kernel_spmd(nc, in_maps, core_ids=list(range(N_CORES)))
    out = np.concatenate([res.results[ci]["out"] for ci in range(N_CORES)],
                         axis=0)
    return out.astype(np.float32)
